# revision 1
# baseline (speedup 1.0000x reference)
"""Trainium2 Bass kernel for ALiBiConformerEncoderLayer (8-core SPMD).

Sharding: data-parallel over sequence windows. Each of the 8 cores handles a
256-token query window (+1 halo token each side) for BOTH batches and ALL 16
heads. alibi is pre-transposed/exponentiated on host so each core streams a
disjoint ~17MB bf16 slice with perfect DMA bursts. No collectives: the
conformer tail is computed locally per window; host concatenates outputs.

Attention layout: scoresT[sk_partition, sq_free] so that
  scoresT = matmul(lhsT=k_roped_T[hd,sk_blk], rhs=q_roped_T[hd,sq])
  attn    = exp(scoresT) * exp_alibiT          (bf16, DVE 4x mode)
  outT    = matmul(lhsT=v_tilde[sk_blk, 17], rhs=attn[sk_blk, sq])  (PSUM acc)
where v_tilde has a 17th ones-column producing the softmax denominator row.
No on-chip transposes anywhere.
"""
import os
import sys
import types
from contextlib import ExitStack

import numpy as np
import ml_dtypes

BF16 = ml_dtypes.bfloat16

# Problem constants (hardcoded; kernel.py must be self-contained)
B, S, D, H, HD = 2, 2048, 256, 16, 16
NCORES = 8

_COMPILED = {}


def _ensure_ntff_hook():
    """Install the axon NTFF profiling hook if the image lacks antenv.axon_hooks."""
    try:
        import antenv.axon_hooks  # noqa: F401
        return
    except ImportError:
        pass
    try:
        from trn_agent_boot.trn_boot import _ntff_profile_via_ctypes
        hook = _ntff_profile_via_ctypes('/opt/axon/libaxon_pjrt.so')
    except Exception:
        hook = None
    mod = types.ModuleType('antenv.axon_hooks')
    mod.get_axon_ntff_profile_hook = lambda: hook
    mod.set_axon_ntff_profile_hook = lambda h: None
    sys.modules['antenv.axon_hooks'] = mod


# ---------------------------------------------------------------------------
# Graph builder
# ---------------------------------------------------------------------------

def build_nc(seq=S, win=None, grp=2, gelu_exact=True):
    """Build the SPMD Bass graph. seq = total sequence, win = per-core window."""
    import concourse.bass as bass
    import concourse.tile as tile
    from concourse import bacc, mybir
    from concourse.bass import ts

    f32 = mybir.dt.float32
    f32r = mybir.dt.float32r
    bf16 = mybir.dt.bfloat16
    AF = mybir.ActivationFunctionType
    OP = mybir.AluOpType

    if win is None:
        win = seq // NCORES
    SQL = win + 2          # query cols incl 1 halo each side
    SQP = (SQL + 3) // 4 * 4  # 8-byte-aligned bf16 stride
    NB = seq // 128        # sk blocks
    NGRP = NB // grp
    NC2 = 2                # D=256 -> 2 partition chunks
    assert NB % grp == 0 and D == 256

    nc = bacc.Bacc(None, target_bir_lowering=False)

    # ---- DRAM parameters (per-core shards supplied via in_maps) ----
    P = {}
    def dram(name, shape, dt):
        P[name] = nc.declare_dram_parameter(name, list(shape), dt, isOutput=False)
        return P[name]

    ealibi = dram("ealibi", [H, 128, NB, SQL], bf16)
    srckv = dram("srckv", [B, D + 1, seq], bf16)      # src[b].T + ones row
    srcqbf = dram("srcqbf", [B, D, SQL], bf16)
    srcqf = dram("srcqf", [B, D, SQL], f32)
    wq2 = dram("wq2", [2, D, D], bf16)                # wqT, wqT swapped-cols
    wk2 = dram("wk2", [2, D, D], bf16)
    wv = dram("wv", [D + 1, D], bf16)                 # wvT + bv row
    ropecs = dram("ropecs", [8, 2, seq], bf16)        # cosT, sinT (k side)
    ropecsq = dram("ropecsq", [8, 2, SQL], bf16)      # 0.25*cos/sin (q side)
    ropem = dram("ropem", [8, 2, 128], bf16)          # Mcos, Msin(signed)
    qkb = dram("qkb", [128, 2, 4], f32)               # bq, bqs, bk, bks
    maskv = dram("maskv", [128, B, seq // 128], f32)  # 1 - mask
    tailw = dram("tailw", [5, D, D], f32r)            # woT pw1T pw2T w1T w2T
    tailv = dram("tailv", [128, 2, 16], f32)
    halom = dram("halom", [1, SQL], f32r)
    pmask = dram("pmask", [128, 8], f32)  # head-slot row masks
    wop = dram("wop", [4 * 128, D], f32r)     # woT rows padded to 32-row head slots
    onesr = dram("onesr", [128, 128], f32r)   # ones (f32r cannot be memset)
    out = nc.declare_dram_parameter("out", [B, D, win], f32, isOutput=True)

    with ExitStack() as top, tile.TileContext(nc) as tc:
        _keep = []
        def ctile(shape, dt, name):
            t, _free = tc.tile(list(shape), dt, name=name)
            _keep.append((t, _free))
            return t

        # ---- const SBUF + DMAs ----
        tw_sb = ctile([128, 5, NC2, D], f32r, "tw_sb")
        tv_sb = ctile([128, 2, 16], f32, "tv_sb")
        qkb_sb = ctile([128, 2, 4], f32, "qkb_sb")
        mv_sb = ctile([128, B, seq // 128], f32, "mv_sb")
        hm_sb = ctile([1, SQL], f32r, "hm_sb")
        qsrcf_sb = ctile([128, B, NC2, SQL], f32, "qsrcf_sb")

        sync = nc.sync
        for w in range(5):
            for cc in range(NC2):
                sync.dma_start(tw_sb[:, w, cc, :], tailw[w, ts(cc, 128), :])
        sync.dma_start(tv_sb[:, :, :], tailv[:, :, :])
        sync.dma_start(qkb_sb[:, :, :], qkb[:, :, :])
        sync.dma_start(mv_sb[:, :, :], maskv[:, :, :])
        sync.dma_start(hm_sb[:, :], halom[:, :])
        for b in range(B):
            for cc in range(NC2):
                sync.dma_start(qsrcf_sb[:, b, cc, :], srcqf[b, ts(cc, 128), :])

        onescol = ctile([128, 16, 1], bf16, "onescol")
        nc.vector.memset(onescol[:, :, :], 1.0)
        eps1 = ctile([1, 1], f32, "eps1")
        nc.vector.memset(eps1[:, :], 1e-5)

        # persistent activation tensors
        kT_sb = ctile([128, B, NC2, seq], bf16, "kT_sb")
        qTv_sb = ctile([128, 8, B, NC2, SQP], bf16, "qTv_sb")
        pm_sb = ctile([128, 8], f32, "pm_sb")
        wop_sb = ctile([128, 4, D], f32r, "wop_sb")
        onesr_sb = ctile([128, 128], f32r, "onesr_sb")
        sync.dma_start(pm_sb[:, :], pmask[:, :])
        for c4 in range(4):
            sync.dma_start(wop_sb[:, c4, :], wop[ts(c4, 128), :])
        sync.dma_start(onesr_sb[:, :], onesr[:, :])
        ones16 = onesr_sb[0:1, 0:16]
        ones32 = onesr_sb[0:1, 0:32]
        ones128 = onesr_sb[:, 0:1]
        onesB = onesr_sb[0:1, :]
        vt_sb = ctile([128, B, NB, 16, 33], bf16, "vt_sb")
        oall_sb = ctile([128, B, 4, SQL], f32r, "oall_sb")
        onum_sb = ctile([33, 16, 2, SQL], bf16, "onum_sb")
        denp_sb = ctile([32, SQL], bf16, "denp_sb")
        recp_sb = ctile([32, SQL], f32r, "recp_sb")
        maskB_sb = ctile([128, SQL], f32, "maskB_sb")

        NF = seq // 512 if seq >= 512 else 1
        FW = min(512, seq)

        # ================= PHASE 1: prologue =================
        with ExitStack() as ph1:
            pro = ph1.enter_context(
                tc.tile_pool(name="pro_psum", bufs=3, space="PSUM"))
            ptmp = ph1.enter_context(tc.tile_pool(name="pro_tmp", bufs=4))
            p1c = ph1.enter_context(tc.tile_pool(name="p1c", bufs=1))

            def p1tile(shape, dt, name):
                return p1c.tile(list(shape), dt, name=name, tag=name)

            wq_sb = p1tile([128, 2, NC2, D], bf16, "wq_sb")
            wk_sb = p1tile([128, 2, NC2, D], bf16, "wk_sb")
            wv_sb = p1tile([128, NC2, D], bf16, "wv_sb")
            wvb_sb = p1tile([1, D], bf16, "wvb_sb")
            rm_sb = p1tile([8, 2, 128], bf16, "rm_sb")
            rcs_sb = p1tile([8, 2, seq], bf16, "rcs_sb")
            rcsq_sb = p1tile([8, 2, SQL], bf16, "rcsq_sb")
            kv_sb = p1tile([128, B, NC2, seq], bf16, "kv_sb")
            kvo_sb = p1tile([1, B, seq], bf16, "kvo_sb")
            qsrc_sb = p1tile([128, B, NC2, SQL], bf16, "qsrc_sb")
            cs_sb = p1tile([128, 2, seq], bf16, "cs_sb")
            csq_sb = p1tile([128, 2, SQL], bf16, "csq_sb")
            qT_sb = p1tile([128, B, NC2, SQL], bf16, "qT_sb")

            for v in range(2):
                for cc in range(NC2):
                    sync.dma_start(wq_sb[:, v, cc, :], wq2[v, ts(cc, 128), :])
                    sync.dma_start(wk_sb[:, v, cc, :], wk2[v, ts(cc, 128), :])
            for cc in range(NC2):
                sync.dma_start(wv_sb[:, cc, :], wv[ts(cc, 128), :])
            sync.dma_start(wvb_sb[:, :], wv[D:D + 1, :])
            sync.dma_start(rm_sb[:, :, :], ropem[:, :, :])
            sync.dma_start(rcs_sb[:, :, :], ropecs[:, :, :])
            sync.dma_start(rcsq_sb[:, :, :], ropecsq[:, :, :])
            for b in range(B):
                for cc in range(NC2):
                    sync.dma_start(kv_sb[:, b, cc, :], srckv[b, ts(cc, 128), :])
                    sync.dma_start(qsrc_sb[:, b, cc, :],
                                   srcqbf[b, ts(cc, 128), :])
                sync.dma_start(kvo_sb[:, b, :], srckv[b, D:D + 1, :])

            # rope broadcast tiles: cs[r, t] = M[., r] rows x cosT/sinT
            for v in range(2):
                for fc in range(NF):
                    pb = pro.tile([128, FW], f32, name="pb", tag="pro")
                    nc.tensor.matmul(pb[:, :], rm_sb[:, v, :],
                                     rcs_sb[:, v, ts(fc, FW)],
                                     start=True, stop=True)
                    nc.scalar.activation(cs_sb[:, v, ts(fc, FW)], pb[:, :],
                                         AF.Copy)
                pbq = pro.tile([128, SQL], f32, name="pbq", tag="pro")
                nc.tensor.matmul(pbq[:, :], rm_sb[:, v, :], rcsq_sb[:, v, :],
                                 start=True, stop=True)
                nc.scalar.activation(csq_sb[:, v, :], pbq[:, :], AF.Copy)

            # halo mask broadcast [1,SQL] -> [128,SQL]
            pmh = pro.tile([128, SQL], f32, name="pmh", tag="pro")
            nc.tensor.matmul(pmh[:, :], onesB, hm_sb[:, :],
                             start=True, stop=True)
            nc.scalar.activation(maskB_sb[:, :], pmh[:, :], AF.Copy)

            # k projection + rope (plain & swapped) -> kT_sb
            for b in range(B):
                for pc in range(NC2):
                    for fc in range(NF):
                        pk = pro.tile([128, FW], f32, name="pk", tag="pro")
                        pks = pro.tile([128, FW], f32, name="pks", tag="pro")
                        for cc in range(NC2):
                            nc.tensor.matmul(
                                pk[:, :], wk_sb[:, 0, cc, ts(pc, 128)],
                                kv_sb[:, b, cc, ts(fc, FW)],
                                start=(cc == 0), stop=(cc == NC2 - 1))
                            nc.tensor.matmul(
                                pks[:, :], wk_sb[:, 1, cc, ts(pc, 128)],
                                kv_sb[:, b, cc, ts(fc, FW)],
                                start=(cc == 0), stop=(cc == NC2 - 1))
                        t1 = ptmp.tile([128, FW], bf16, name="t1", tag="ptmp")
                        t2 = ptmp.tile([128, FW], bf16, name="t2", tag="ptmp")
                        nc.vector.scalar_tensor_tensor(
                            t1[:, :], pk[:, :], qkb_sb[:, pc, 2:3],
                            cs_sb[:, 0, ts(fc, FW)], op0=OP.add, op1=OP.mult)
                        nc.vector.scalar_tensor_tensor(
                            t2[:, :], pks[:, :], qkb_sb[:, pc, 3:4],
                            cs_sb[:, 1, ts(fc, FW)], op0=OP.add, op1=OP.mult)
                        nc.vector.tensor_add(kT_sb[:, b, pc, ts(fc, FW)],
                                             t1[:, :], t2[:, :])

            # q projection + rope (0.25-scaled cos/sin) -> qT_sb
            for b in range(B):
                for pc in range(NC2):
                    pq = pro.tile([128, SQL], f32, name="pq", tag="pro")
                    pqs = pro.tile([128, SQL], f32, name="pqs", tag="pro")
                    for cc in range(NC2):
                        nc.tensor.matmul(
                            pq[:, :], wq_sb[:, 0, cc, ts(pc, 128)],
                            qsrc_sb[:, b, cc, :],
                            start=(cc == 0), stop=(cc == NC2 - 1))
                        nc.tensor.matmul(
                            pqs[:, :], wq_sb[:, 1, cc, ts(pc, 128)],
                            qsrc_sb[:, b, cc, :],
                            start=(cc == 0), stop=(cc == NC2 - 1))
                    t1 = ptmp.tile([128, SQL], bf16, name="t1q", tag="ptmp")
                    t2 = ptmp.tile([128, SQL], bf16, name="t2q", tag="ptmp")
                    nc.vector.scalar_tensor_tensor(
                        t1[:, :], pq[:, :], qkb_sb[:, pc, 0:1],
                        csq_sb[:, 0, :], op0=OP.add, op1=OP.mult)
                    nc.vector.scalar_tensor_tensor(
                        t2[:, :], pqs[:, :], qkb_sb[:, pc, 1:2],
                        csq_sb[:, 1, :], op0=OP.add, op1=OP.mult)
                    nc.vector.tensor_add(qT_sb[:, b, pc, :], t1[:, :], t2[:, :])
                    for sv in range(8):
                        nc.vector.tensor_scalar(
                            qTv_sb[:, sv, b, pc, 0:SQL], qT_sb[:, b, pc, :],
                            pm_sb[:, sv:sv + 1], None, op0=OP.mult)

            # v projection -> vt_sb [sk, b, blk, h, 33]: v cols 0:16, ones col 32
            nc.gpsimd.memset(vt_sb[:, :, :, :, 16:32], 0.0)
            for b in range(B):
                for tcn in range(NB):
                    pv = pro.tile([128, 16, 16], f32, name="pv", tag="pro")
                    for cc in range(NC2):
                        nc.tensor.matmul(pv[:, :, :], kv_sb[:, b, cc, ts(tcn, 128)],
                                         wv_sb[:, cc, :],
                                         start=(cc == 0), stop=False)
                    nc.tensor.matmul(pv[:, :, :], kvo_sb[:, b, ts(tcn, 128)],
                                     wvb_sb[:, :], start=False, stop=True)
                    nc.vector.tensor_scalar(
                        vt_sb[:, b, tcn, 0:16, 0:16], pv[:, :, :],
                        mv_sb[:, b, tcn:tcn + 1], None, op0=OP.mult)
                    nc.vector.tensor_scalar(
                        vt_sb[:, b, tcn, 0:16, 32:33], onescol[:, :, :],
                        mv_sb[:, b, tcn:tcn + 1], None, op0=OP.mult)

        # ================= PHASE 2: attention =================
        with ExitStack() as ph2:
            peal = ph2.enter_context(tc.tile_pool(name="peal", bufs=3))
            pscore = ph2.enter_context(
                tc.tile_pool(name="pscore", bufs=2, space="PSUM"))
            po_pool = ph2.enter_context(
                tc.tile_pool(name="po", bufs=2, space="PSUM"))
            pexp = ph2.enter_context(tc.tile_pool(name="pexp", bufs=3))
            pattn = ph2.enter_context(tc.tile_pool(name="pattn", bufs=3))

            for cc in range(NC2):
                for pr in range(4):
                    h0 = cc * 8 + 2 * pr
                    eal = peal.tile([128, NB, 2, SQP], bf16, name="eal",
                                    tag="eal")
                    sync.dma_start(eal[:, :, 0, 0:SQL], ealibi[h0, :, :, :])
                    sync.dma_start(eal[:, :, 1, 0:SQL],
                                   ealibi[h0 + 1, :, :, :])
                    for b in range(B):
                        po2 = po_pool.tile([33, 2, 512], f32, name="po2",
                                           tag="po")
                        for blk in range(NB):
                            sc = pscore.tile([128, 2, 512], f32, name="sc",
                                             tag="sc")
                            for j in range(2):
                                nc.tensor.matmul(
                                    sc[:, j, 0:SQL],
                                    kT_sb[:, b, cc, ts(blk, 128)],
                                    qTv_sb[:, 2 * pr + j, b, cc, 0:SQL],
                                    start=True, stop=True)
                            ex = pexp.tile([128, 2, SQP], bf16, name="ex",
                                           tag="ex")
                            nc.scalar.activation(ex[:, :, 0:SQL],
                                                 sc[:, :, 0:SQL], AF.Exp)
                            at = pattn.tile([128, 2, SQP], bf16, name="at",
                                            tag="at")
                            mul_eng = nc.vector if blk % 2 == 0 else nc.gpsimd
                            mul_eng.tensor_mul(at[:, :, 0:SQL],
                                               ex[:, :, 0:SQL],
                                               eal[:, blk, :, 0:SQL])
                            for j in range(2):
                                nc.tensor.matmul(
                                    po2[:, j, 0:SQL],
                                    vt_sb[:, b, blk, h0 + j, :],
                                    at[:, j, 0:SQL],
                                    start=(blk == 0), stop=(blk == NB - 1),
                                    skip_group_check=True)
                        pidx = (cc * 4 + pr) * B + b
                        nc.vector.tensor_copy(onum_sb[:, pidx, :, :],
                                              po2[:, :, 0:SQL])

        # ---- batched softmax division epilogue ----
        with ExitStack() as ph2b:
            pbc = ph2b.enter_context(
                tc.tile_pool(name="pbc", bufs=4, space="PSUM"))
            ptb = ph2b.enter_context(tc.tile_pool(name="ptb", bufs=1))
            lntmp = ptb.tile([32, SQL], f32, name="lntmp", tag="lt")
            recf_sb = ptb.tile([1, 32, SQL], f32r, name="recf", tag="rf")
            # dens: onum row 32 -> partition-stacked [64, SQL]
            nc.sync.dma_start(denp_sb[:, :], onum_sb[32:33, :, :, :])
            nc.scalar.activation(lntmp[:, :], denp_sb[:, :], AF.Ln)
            nc.scalar.activation(recp_sb[:, :], lntmp[:, :], AF.Exp,
                                 scale=-1.0)
            nc.sync.dma_start(recf_sb[:, :, :], recp_sb[:, :])
            for cc in range(NC2):
                for pr in range(4):
                    for b in range(B):
                        pidx = (cc * 4 + pr) * B + b
                        for j in range(2):
                            h = cc * 8 + 2 * pr + j
                            rb = pbc.tile([32, SQL], f32, name="rb", tag="rb")
                            nc.tensor.matmul(
                                rb[:, :], ones32,
                                recf_sb[0:1, 2 * pidx + j, :],
                                start=True, stop=True)
                            ro2 = (h % 4) * 32
                            nc.vector.tensor_mul(
                                oall_sb[ro2:ro2 + 32, b, h // 4, :],
                                onum_sb[0:32, pidx, j, :], rb[:, :])

        # ================= PHASE 3: conformer tail =================
        with ExitStack() as ph3:
            ptp = ph3.enter_context(
                tc.tile_pool(name="tail_psum", bufs=6, space="PSUM"))
            pt = ph3.enter_context(tc.tile_pool(name="tail_sb", bufs=12))
            pt1 = ph3.enter_context(tc.tile_pool(name="tail_sb1", bufs=8))

            def tv(pc, i):
                return tv_sb[:, pc, i:i + 1]

            def gelu_act(dst, psum_in, bias_ap):
                if gelu_exact:
                    nc.scalar.activation(dst[:, :], psum_in[:, :], AF.Gelu,
                                         bias=bias_ap)
                    return
                F2 = dst.shape[-1]
                y = pt.tile([128, F2], f32, name="gy", tag="pt")
                nc.vector.tensor_scalar(y[:, :], psum_in[:, :], bias_ap,
                                        None, op0=OP.add)
                x3 = pt.tile([128, F2], f32, name="gx3", tag="pt")
                nc.scalar.activation(x3[:, :], y[:, :], AF.Square)
                nc.vector.tensor_mul(x3[:, :], x3[:, :], y[:, :])
                u = pt.tile([128, F2], f32, name="gu", tag="pt")
                nc.vector.scalar_tensor_tensor(
                    u[:, :], x3[:, :], 0.044715, y[:, :],
                    op0=OP.mult, op1=OP.add)
                t = pt.tile([128, F2], f32, name="gt", tag="pt")
                nc.scalar.activation(t[:, :], u[:, :], AF.Tanh,
                                     scale=0.7978845608028654)
                v = pt.tile([128, F2], f32, name="gv", tag="pt")
                nc.vector.tensor_scalar(v[:, :], t[:, :], 1.0, None,
                                        op0=OP.add)
                nc.vector.scalar_tensor_tensor(
                    dst[:, :], y[:, :], 0.5, v[:, :],
                    op0=OP.mult, op1=OP.mult)

            def layernorm(xin, F, gi, bi, odt=f32r):
                """LN over channel dim (256 = partitions across 2 chunks)."""
                ps = ptp.tile([1, F], f32, name="ln_ps", tag="tp")
                ps2 = ptp.tile([1, F], f32, name="ln_ps2", tag="tp")
                for pc in range(NC2):
                    nc.tensor.matmul(ps[:, :], ones128, xin[pc][:, 0:F],
                                     start=(pc == 0), stop=(pc == NC2 - 1))
                sqs = []
                for pc in range(NC2):
                    sq = pt1.tile([128, F], f32r, name="ln_sq", tag="pt1")
                    nc.scalar.activation(sq[:, :], xin[pc][:, 0:F], AF.Square)
                    sqs.append(sq)
                for pc in range(NC2):
                    nc.tensor.matmul(ps2[:, :], ones128, sqs[pc][:, :],
                                     start=(pc == 0), stop=(pc == NC2 - 1))
                mean = pt1.tile([1, F], f32r, name="ln_mean", tag="pt1")
                nc.scalar.activation(mean[:, :], ps[:, :], AF.Copy,
                                     scale=1.0 / D)
                m2 = pt1.tile([1, F], f32, name="ln_m2", tag="pt1")
                nc.scalar.activation(m2[:, :], mean[:, :], AF.Square)
                var = pt1.tile([1, F], f32, name="ln_var", tag="pt1")
                nc.vector.scalar_tensor_tensor(
                    var[:, :], ps2[:, :], 1.0 / D, m2[:, :],
                    op0=OP.mult, op1=OP.subtract)
                sd = pt1.tile([1, F], f32, name="ln_sd", tag="pt1")
                nc.scalar.activation(sd[:, :], var[:, :], AF.Sqrt, bias=eps1[:, :])
                rstd = pt1.tile([1, F], f32r, name="ln_rstd", tag="pt1")
                with nc.allow_low_precision(reason="f32r is fp32-width"):
                    nc.vector.reciprocal(rstd[:, :], sd[:, :])
                pmb = ptp.tile([128, F], f32, name="ln_pmb", tag="tp")
                nc.tensor.matmul(pmb[:, :], onesB, mean[:, :],
                                 start=True, stop=True)
                prb = ptp.tile([128, F], f32, name="ln_prb", tag="tp")
                nc.tensor.matmul(prb[:, :], onesB, rstd[:, :],
                                 start=True, stop=True)
                outs = []
                for pc in range(NC2):
                    t = pt.tile([128, F], f32, name="ln_t", tag="pt")
                    nc.vector.tensor_sub(t[:, :], xin[pc][:, 0:F], pmb[:, :])
                    t2 = pt.tile([128, F], f32, name="ln_t2", tag="pt")
                    nc.vector.tensor_mul(t2[:, :], t[:, :], prb[:, :])
                    o = pt.tile([128, F], odt, name="ln_o", tag="pt")
                    nc.scalar.activation(o[:, :], t2[:, :], AF.Identity,
                                         bias=tv(pc, bi), scale=tv(pc, gi))
                    outs.append(o)
                return outs

            def mm4(widx, rhs_tiles, F, name):
                """out[pc] psum = W[widx] @ rhs (256x256 fp32r matmul)."""
                outs = []
                for pc in range(NC2):
                    p = ptp.tile([128, F], f32, name=name, tag="tp")
                    for cc in range(NC2):
                        nc.tensor.matmul(p[:, :],
                                         tw_sb[:, widx, cc, ts(pc, 128)],
                                         rhs_tiles[cc][:, 0:F],
                                         start=(cc == 0), stop=(cc == NC2 - 1))
                    outs.append(p)
                return outs

            for b in range(B):
                # o-projection + residual
                oproj = []
                for pc in range(NC2):
                    px = ptp.tile([128, SQL], f32, name="px", tag="tp")
                    for c4 in range(4):
                        nc.tensor.matmul(
                            px[:, :],
                            wop_sb[:, c4, ts(pc, 128)],
                            oall_sb[:, b, c4, :],
                            start=(c4 == 0), stop=(c4 == 3))
                    x1p = pt.tile([128, SQL], f32r, name="x1p", tag="pt")
                    nc.vector.scalar_tensor_tensor(
                        x1p[:, :], px[:, :], tv(pc, 0),
                        qsrcf_sb[:, b, pc, :], op0=OP.add, op1=OP.add)
                    oproj.append(x1p)
                x1 = layernorm(oproj, SQL, 1, 2)
                c0 = layernorm(x1, SQL, 3, 4)
                # pw1 + gelu + halo mask
                cp = mm4(1, c0, SQL, "pc1")
                cm = []
                for pc in range(NC2):
                    cg = pt.tile([128, SQL], f32, name="cg", tag="pt")
                    gelu_act(cg, cp[pc], tv(pc, 5))
                    cmt = pt.tile([128, SQL], f32r, name="cmt", tag="pt")
                    nc.vector.tensor_mul(cmt[:, :], cg[:, :], maskB_sb[:, :])
                    cm.append(cmt)
                # depthwise conv (3 taps) + BN + hardswish
                hsw = []
                for pc in range(NC2):
                    a1 = pt.tile([128, win], f32, name="a1", tag="pt")
                    nc.vector.tensor_scalar(a1[:, :], cm[pc][:, 1:win + 1],
                                            tv(pc, 7), None, op0=OP.mult)
                    a2 = pt.tile([128, win], f32, name="a2", tag="pt")
                    nc.vector.scalar_tensor_tensor(
                        a2[:, :], cm[pc][:, 0:win], tv(pc, 6), a1[:, :],
                        op0=OP.mult, op1=OP.add)
                    a3 = pt.tile([128, win], f32, name="a3", tag="pt")
                    nc.vector.scalar_tensor_tensor(
                        a3[:, :], cm[pc][:, 2:win + 2], tv(pc, 8), a2[:, :],
                        op0=OP.mult, op1=OP.add)
                    bn = pt.tile([128, win], f32, name="bn", tag="pt")
                    nc.scalar.activation(bn[:, :], a3[:, :], AF.Identity,
                                         bias=tv(pc, 10), scale=tv(pc, 9))
                    h1 = pt.tile([128, win], f32, name="h1", tag="pt")
                    nc.vector.tensor_scalar(h1[:, :], bn[:, :], 3.0, 6.0,
                                            op0=OP.add, op1=OP.min)
                    h2 = pt.tile([128, win], f32, name="h2", tag="pt")
                    nc.vector.tensor_scalar(h2[:, :], h1[:, :], 0.0, None,
                                            op0=OP.max)
                    hst = pt.tile([128, win], f32r, name="hst", tag="pt")
                    nc.vector.scalar_tensor_tensor(
                        hst[:, :], bn[:, :], 1.0 / 6.0, h2[:, :],
                        op0=OP.mult, op1=OP.mult)
                    hsw.append(hst)
                # pw2
                p2 = mm4(2, hsw, win, "p2")
                x2 = []
                for pc in range(NC2):
                    x2t = pt.tile([128, win], f32r, name="x2t", tag="pt")
                    nc.vector.tensor_scalar(x2t[:, :], p2[pc][:, :],
                                            tv(pc, 11), None, op0=OP.add)
                    x2.append(x2t)
                # FFN
                p3 = mm4(3, x2, win, "p3")
                gg = []
                for pc in range(NC2):
                    g1 = pt.tile([128, win], f32r, name="g1", tag="pt")
                    gelu_act(g1, p3[pc], tv(pc, 12))
                    gg.append(g1)
                p4 = mm4(4, gg, win, "p4")
                x3 = []
                for pc in range(NC2):
                    x3t = pt.tile([128, win], f32r, name="x3t", tag="pt")
                    nc.vector.scalar_tensor_tensor(
                        x3t[:, :], p4[pc][:, :], tv(pc, 13), x2[pc][:, :],
                        op0=OP.add, op1=OP.add)
                    x3.append(x3t)
                xo = layernorm(x3, win, 14, 15, odt=f32)
                for pc in range(NC2):
                    sync.dma_start(out[b, ts(pc, 128), :], xo[pc][:, :])

        for _t, _free in reversed(_keep):
            _free()

    nc.compile()
    return nc


# ---------------------------------------------------------------------------
# Host-side input prep (sharding)
# ---------------------------------------------------------------------------

def host_prep(inputs, seq=S, win=None, cores=NCORES):
    """Build per-core in_maps from full inputs."""
    if win is None:
        win = seq // cores
    SQL = win + 2
    NB = seq // 128
    f32 = np.float32

    src = np.asarray(inputs["src"], f32)
    alibi = np.asarray(inputs["alibi_bias"], f32)
    pos_emb = np.asarray(inputs["pos_emb"], f32)
    mask = np.asarray(inputs["mask"])

    b_, s_, d_ = src.shape
    NB_ = seq // 128
    ealibi_full = np.exp(alibi, dtype=f32)  # [H, S, S]

    # swap perm within each head: j -> (j+8)%16; sign -1 for j<8, +1 for j>=8
    jj = np.arange(D)
    swap = (jj // HD) * HD + (jj % HD + HD // 2) % HD

    cos = np.cos(pos_emb).astype(f32)  # [S, 8]
    sin = np.sin(pos_emb).astype(f32)

    wq, wk, wvm = [np.asarray(inputs[k], f32) for k in ("wq", "wk", "wv")]
    bq, bk, bv = [np.asarray(inputs[k], f32) for k in ("bq", "bk", "bv")]

    wq2 = np.ascontiguousarray(np.stack([wq.T, wq.T[:, swap]])).astype(BF16)
    wk2 = np.ascontiguousarray(np.stack([wk.T, wk.T[:, swap]])).astype(BF16)
    wv2 = np.concatenate([wvm.T, bv[None, :]], 0).astype(BF16)

    # [8, 2, S]: cosT, sinT interleaved on axis 1
    ropecs = np.ascontiguousarray(
        np.stack([cos.T, sin.T], 1)).astype(BF16)
    # Mcos[i, v, r]: cos-select (r%8==i); Msin adds sign by half
    r = np.arange(128)
    mc = (r[None, :] % 8 == np.arange(8)[:, None]).astype(f32)
    sgn_r = np.where((r % HD) < HD // 2, -1.0, 1.0).astype(f32)
    ropem = np.ascontiguousarray(
        np.stack([mc, mc * sgn_r[None, :]], 1)).astype(BF16)

    qkbv = np.stack([bq, bq[swap], bk, bk[swap]], -1)  # [D, 4]
    qkbv = np.ascontiguousarray(
        qkbv.reshape(2, 128, 4).transpose(1, 0, 2)).astype(f32)

    maskvec = (1.0 - mask.astype(f32))  # [B, S]
    maskv = np.ascontiguousarray(
        maskvec.reshape(b_, NB_, 128).transpose(2, 0, 1)).astype(f32)

    # tail weights / vectors
    wo, pw1, pw2, w1m, w2m = [np.asarray(inputs[k], f32)
                              for k in ("wo", "pw1_w", "pw2_w", "w1", "w2")]
    tailw = np.ascontiguousarray(
        np.stack([wo.T, pw1.T, pw2.T, w1m.T, w2m.T])).astype(f32)
    dww = np.asarray(inputs["dw_w"], f32)  # [D, 1, 3]
    sbn = (np.asarray(inputs["bn_g"], f32) /
           np.sqrt(np.asarray(inputs["bn_var"], f32) + 1e-5))
    tbn = ((np.asarray(inputs["dw_b"], f32) -
            np.asarray(inputs["bn_mean"], f32)) * sbn +
           np.asarray(inputs["bn_b"], f32))
    vecs = [inputs["bo"], inputs["n1_g"], inputs["n1_b"], inputs["ln_g"],
            inputs["ln_b"], inputs["pw1_b"], dww[:, 0, 0], dww[:, 0, 1],
            dww[:, 0, 2], sbn, tbn, inputs["pw2_b"], inputs["b1"],
            inputs["b2"], inputs["n2_g"], inputs["n2_b"]]
    tailv = np.stack([np.asarray(v, f32) for v in vecs], -1)  # [D, 16]
    tailv = np.ascontiguousarray(
        tailv.reshape(2, 128, 16).transpose(1, 0, 2)).astype(f32)

    srckv = np.concatenate(
        [src.transpose(0, 2, 1), np.ones((b_, 1, s_), f32)], 1).astype(BF16)

    rr = np.arange(128)
    pmaskh = np.stack([(rr // 16 == j).astype(f32)
                       for j in range(8)], -1)  # [128, 8]

    wop = np.zeros((512, D), f32)
    r512 = np.arange(512)
    real = (r512 % 32) < 16
    dsrc = (r512 // 32) * 16 + (r512 % 32)
    wop[real, :] = wo.T[dsrc[real], :]

    in_maps = []
    for c in range(cores):
        q0 = c * win - 1  # may be -1
        # q-window slice with zero padding
        sq = np.zeros((b_, SQL, d_), f32)
        lo, hi = max(q0, 0), min(q0 + SQL, s_)
        sq[:, lo - q0:hi - q0, :] = src[:, lo:hi, :]
        srcqT = sq.transpose(0, 2, 1)  # [B, D, SQL]

        ea = np.ones((H, SQL, s_), f32)
        ea[:, lo - q0:hi - q0, :] = ealibi_full[:, lo:hi, :]
        # -> [H, sk, sq] -> [H, 128p, NB, SQL]
        eaT = ea.transpose(0, 2, 1).reshape(H, NB_, 128, SQL)
        eaT = np.ascontiguousarray(eaT.transpose(0, 2, 1, 3)).astype(BF16)

        csq = np.zeros((8, 2, SQL), f32)
        csq[:, 0, lo - q0:hi - q0] = 0.25 * cos[lo:hi, :].T
        csq[:, 1, lo - q0:hi - q0] = 0.25 * sin[lo:hi, :].T

        halo = np.ones((1, SQL), f32)
        if q0 < 0:
            halo[0, 0] = 0.0
        if q0 + SQL > s_:
            halo[0, SQL - 1] = 0.0

        in_maps.append({
            "ealibi": eaT,
            "srckv": srckv,
            "srcqbf": srcqT.astype(BF16),
            "srcqf": srcqT.astype(f32),
            "wq2": wq2, "wk2": wk2, "wv": wv2,
            "ropecs": ropecs,
            "ropecsq": csq.astype(BF16),
            "ropem": ropem,
            "qkb": qkbv,
            "maskv": maskv,
            "tailw": tailw,
            "tailv": tailv,
            "halom": halo,
            "pmask": pmaskh,
            "wop": wop,
            "onesr": np.ones((128, 128), f32),
        })
    return in_maps


def kernel(**inputs) -> np.ndarray:
    _ensure_ntff_hook()
    from concourse.bass_utils import run_bass_kernel_spmd

    seq = inputs["src"].shape[1]
    win = seq // NCORES
    key = (seq, win)
    if key not in _COMPILED:
        _COMPILED[key] = build_nc(seq=seq, win=win)
    nc = _COMPILED[key]

    in_maps = host_prep(inputs, seq=seq, win=win)
    trace = os.environ.get("KERNEL_TRACE", "0") == "1"
    res = run_bass_kernel_spmd(nc, in_maps, core_ids=list(range(NCORES)),
                               trace=trace)
    kernel.last_result = res

    b_, s_, d_ = inputs["src"].shape
    full = np.empty((b_, s_, d_), np.float32)
    for c in range(NCORES):
        o = res.results[c]["out"]  # [B, D, win]
        full[:, c * win:(c + 1) * win, :] = o.transpose(0, 2, 1)
    return full

